# revision 1
# baseline (speedup 1.0000x reference)
"""Trainium2 Bass kernel for nn_DistHead (block-diagonal molecule attention).

out = softmax_blockdiag(Q K^T / sqrt(H)) * exp(-invr0 * cdist(Z, Z)) @ V
with Q/K/V = X @ W{q,k,v}^T, block-diagonal over 128 molecules of 64 atoms.

Sharding: 16 whole molecules (1024 rows) per core across 8 cores —
perfectly parallel, zero cross-core communication.

Key tricks:
- Block-diagonal mask folded into the score matmul: Q^T/K^T get two extra
  contraction rows (+-16 patterns) that add exactly 0 for same-molecule
  pairs and -512 for cross-molecule pairs inside a 128-row tile, so
  exp() underflows off-block scores to exactly 0. No mask ops at all.
- Pairwise distance^2 via one K=5 matmul using augmented coordinates
  [z2, 1, -2z] x [1, z2, z].
- All elementwise work batched into single [128, 1024]-wide ops.

Self-contained: hardcodes shapes from the problem spec; only imports
concourse from /opt/trn_rl_repo.
"""

import sys

if "/opt/trn_rl_repo" not in sys.path:
    sys.path.insert(0, "/opt/trn_rl_repo")

import numpy as np

N, E, H = 8192, 256, 64          # atoms, embedding, head size
NSEG, SEG = 128, 64              # molecules, atoms per molecule
NCORES = 8
RPC = N // NCORES                # rows per core (1024 = 16 molecules)
NT = RPC // 128                  # 128-row tiles per core (2 molecules each)
EC = E // 128                    # embedding chunks of 128

_cache = {}


def _build_nc():
    import concourse.bacc as bacc
    import concourse.tile as tile
    from concourse import mybir

    f32 = mybir.dt.float32
    f16 = mybir.dt.float16
    AF = mybir.ActivationFunctionType
    X_AX = mybir.AxisListType.X

    nc = bacc.Bacc(None, target_bir_lowering=False, debug=False)

    # fp16 operands for all matmuls except the distance gram matmul (kept
    # fp32: catastrophic cancellation for nearby atoms needs the mantissa).
    xt_d = nc.dram_tensor("xt", [128, EC, RPC], f16, kind="ExternalInput")
    ident_d = nc.dram_tensor("ident", [128, 128], f16, kind="ExternalInput")
    w_d = nc.dram_tensor("w", [128, 6, H], f16, kind="ExternalInput")
    zat_d = nc.dram_tensor("zat", [5, RPC], f32, kind="ExternalInput")
    zbt_d = nc.dram_tensor("zbt", [5, RPC], f32, kind="ExternalInput")
    qaug_d = nc.dram_tensor("qaug", [2, RPC], f16, kind="ExternalInput")
    kaug_d = nc.dram_tensor("kaug", [2, RPC], f16, kind="ExternalInput")
    y_d = nc.dram_tensor("y", [RPC, H], f32, kind="ExternalOutput")

    with tile.TileContext(nc) as tc:
        with (
            tc.tile_pool(name="consts", bufs=1) as consts,
            tc.tile_pool(name="sb", bufs=4) as sb,
            tc.tile_pool(name="sm", bufs=2) as sm,
            tc.tile_pool(name="wide", bufs=1) as wide,
            tc.tile_pool(name="psm", bufs=4, space="PSUM") as psm,
            tc.tile_pool(name="psb", bufs=1, space="PSUM") as psb,
        ):
            zat = consts.tile([5, RPC], f32, tag="zat")
            nc.sync.dma_start(out=zat, in_=zat_d[:, :])
            zbt = consts.tile([5, RPC], f32, tag="zbt")
            nc.sync.dma_start(out=zbt, in_=zbt_d[:, :])

            w_sb = consts.tile([128, 6, H], f16, tag="w")
            nc.sync.dma_start(out=w_sb, in_=w_d[:, :, :])
            xt = consts.tile([128, EC, RPC], f16, tag="xt")
            for c in range(EC):
                nc.sync.dma_start(out=xt[:, c, :], in_=xt_d[:, c, :])

            # Distance^2 matmuls + decay chain first: they only need the tiny
            # zat/zbt DMAs, so the Sqrt/Exp ACT table loads and the whole
            # dist pipeline overlap the xt load and QK phase.
            d_ps = psb.tile([128, NT, 128], f32, tag="d")
            with tc.high_priority():
                for t in range(NT):
                    rt = slice(t * 128, (t + 1) * 128)
                    nc.tensor.matmul(d_ps[:, t, :], lhsT=zat[:, rt], rhs=zbt[:, rt], start=True, stop=True)
                dist = wide.tile([128, NT, 128], f32, tag="dist")
                nc.vector.tensor_scalar_max(out=dist, in0=d_ps, scalar1=0.0)
                nc.scalar.activation(out=dist, in_=dist, func=AF.Sqrt)
                disth = wide.tile([128, NT, 128], f16, tag="disth")
                nc.scalar.activation(out=disth, in_=dist, func=AF.Exp, scale=-1.0)

            ident = consts.tile([128, 128], f16, tag="ident")
            nc.scalar.dma_start(out=ident, in_=ident_d[:, :])

            # Q^T / K^T with two augmented mask rows each: [66, RPC].
            qt = consts.tile([H + 2, RPC], f16, tag="qt")
            kt = consts.tile([H + 2, RPC], f16, tag="kt")
            nc.scalar.dma_start(out=qt[H : H + 2, :], in_=qaug_d[:, :])
            nc.scalar.dma_start(out=kt[H : H + 2, :], in_=kaug_d[:, :])
            for iw, dst in ((0, qt), (1, kt)):
                for h in range(RPC // 512):
                    p = psm.tile([H, 512], f32, tag="mi")
                    for c in range(EC):
                        nc.tensor.matmul(
                            p,
                            lhsT=w_sb[:, 2 * iw + c, :],
                            rhs=xt[:, c, h * 512 : (h + 1) * 512],
                            start=(c == 0),
                            stop=(c == EC - 1),
                        )
                    nc.vector.tensor_copy(out=dst[0:H, h * 512 : (h + 1) * 512], in_=p)

            # Scores for all NT tiles; each matmul's 128-col slice stays
            # inside one psum bank.
            s_ps = psb.tile([128, NT, 128], f32, tag="s")
            for t in range(NT):
                rt = slice(t * 128, (t + 1) * 128)
                nc.tensor.matmul(s_ps[:, t, :], lhsT=qt[:, rt], rhs=kt[:, rt], start=True, stop=True)

            # exp(S): off-block entries are ~-504 -> exactly 0 (in fp16 too),
            # so dense row sums and the dense PV matmul are already correct.
            # Split into halves so tiles 0-3's transposes start while the
            # second half is still in the exp/mul chain.
            HF = NT // 2
            e = wide.tile([128, NT, 128], f16, tag="e")
            wei = wide.tile([128, NT, 128], f16, tag="wei")
            rowsum = sm.tile([128, NT], f32, tag="rowsum")
            rinv = sm.tile([128, NT], f32, tag="rinv")
            for hh in range(2):
                hs = slice(hh * HF, (hh + 1) * HF)
                nc.scalar.activation(out=e[:, hs, :], in_=s_ps[:, hs, :], func=AF.Exp)
                nc.vector.tensor_mul(out=wei[:, hs, :], in0=e[:, hs, :], in1=disth[:, hs, :])
                nc.vector.reduce_sum(out=rowsum[:, hs], in_=e[:, hs, :], axis=X_AX)
                nc.vector.reciprocal(out=rinv[:, hs], in_=rowsum[:, hs])

            # V emitted after the elementwise chain: PE runs it inside the
            # bubble while ACT/DVE work. [128, NT, H] row-major.
            v_sb = consts.tile([128, NT, H], f16, tag="v")
            for t in range(NT):
                p = psm.tile([128, H], f32, tag="mi")
                for c in range(EC):
                    nc.tensor.matmul(
                        p,
                        lhsT=xt[:, c, t * 128 : (t + 1) * 128],
                        rhs=w_sb[:, 4 + c, :],
                        start=(c == 0),
                        stop=(c == EC - 1),
                    )
                nc.vector.tensor_copy(out=v_sb[:, t, :], in_=p)

            # Transpose + PV, software-pipelined so the PE never waits on the
            # psum->sbuf hop: all transposes can run back to back.
            o_all = sb.tile([128, NT, H], f32, tag="o_all")
            wt_ps = [None] * NT
            wt_sb = [None] * NT
            for t in range(NT):
                wt_ps[t] = psm.tile([128, 128], f16, name=f"wt_ps{t}", tag="mi")
                nc.tensor.transpose(wt_ps[t], wei[:, t, :], ident)
                wt_sb[t] = sb.tile([128, 128], f16, name=f"wt_sb{t}", tag="wt")
                nc.vector.tensor_copy(out=wt_sb[t], in_=wt_ps[t])
                if t >= 1:
                    _pv(nc, psm, o_all, wt_sb, v_sb, rinv, t - 1)
            _pv(nc, psm, o_all, wt_sb, v_sb, rinv, NT - 1)
            y_r = y_d.rearrange("(t p) h -> p t h", p=128)
            nc.sync.dma_start(out=y_r[:, 0:HF, :], in_=o_all[:, 0:HF, :])
            nc.sync.dma_start(out=y_r[:, HF:NT, :], in_=o_all[:, HF:NT, :])

    nc.compile()
    return nc


def _pv(nc, psm, o_all, wt_sb, v_sb, rinv, t):
    from concourse import mybir

    f32 = mybir.dt.float32
    o_ps = psm.tile([128, H], f32, tag="mi")
    nc.tensor.matmul(o_ps, lhsT=wt_sb[t], rhs=v_sb[:, t, :], start=True, stop=True)
    nc.vector.tensor_scalar_mul(out=o_all[:, t, :], in0=o_ps, scalar1=rinv[:, t : t + 1])


def _get_nc():
    if "nc" not in _cache:
        _cache["nc"] = _build_nc()
    return _cache["nc"]


def _prepare_in_maps(X, Z, Wk, Wq, Wv, invr0):
    X = np.ascontiguousarray(X, dtype=np.float32)
    Z = np.ascontiguousarray(Z, dtype=np.float32)
    # [128, EC, N] fp16: partition p, chunk c -> X^T row c*128+p.
    xt_full = np.ascontiguousarray(
        X.T.reshape(EC, 128, N).transpose(1, 0, 2).astype(np.float16)
    )

    # invr0 folded into the distance operands: dist_psum = invr0^2 * d2,
    # so after sqrt the decay is exp(-1.0 * x).
    inv = np.float32(np.asarray(invr0).reshape(-1)[0])
    z2 = np.sum(Z * Z, axis=-1)
    ones = np.ones(N, dtype=np.float32)
    zt = np.ascontiguousarray(Z.T)
    zat_full = (inv * np.concatenate([z2[None], ones[None], -2.0 * zt], axis=0)).astype(np.float32)
    zbt_full = (inv * np.concatenate([ones[None], z2[None], zt], axis=0)).astype(np.float32)

    scale = np.float32(H) ** np.float32(-0.5)
    # [128, 6, H]: chunks [wq0, wq1, wk0, wk1, wv0, wv1].
    w_parts = [
        (Wq.T * scale).astype(np.float32).reshape(EC, 128, H),
        Wk.T.astype(np.float32).reshape(EC, 128, H),
        Wv.T.astype(np.float32).reshape(EC, 128, H),
    ]
    w_full = np.ascontiguousarray(
        np.stack([p[c] for p in w_parts for c in range(EC)], axis=1).astype(np.float16)
    )

    # Mask rows: same-molecule pairs within a 128-row tile add exactly 0,
    # cross-molecule pairs add -512 (256 and +-16 are exact in fp16).
    sig = np.where((np.arange(RPC) % 128) < SEG, 16.0, -16.0).astype(np.float16)
    ones_r = np.ones(RPC, dtype=np.float16)
    qaug = np.ascontiguousarray(np.stack([ones_r, sig]).astype(np.float16))
    kaug = np.ascontiguousarray(np.stack([-256.0 * ones_r, sig]).astype(np.float16))

    in_maps = []
    for d in range(NCORES):
        s, e = d * RPC, (d + 1) * RPC
        in_maps.append(
            {
                "xt": np.ascontiguousarray(xt_full[:, :, s:e]),
                "zat": np.ascontiguousarray(zat_full[:, s:e]),
                "zbt": np.ascontiguousarray(zbt_full[:, s:e]),
                "w": w_full,
                "ident": np.eye(128, dtype=np.float16),
                "qaug": qaug,
                "kaug": kaug,
            }
        )
    return in_maps


def _run(in_maps, trace=False, **kwargs):
    from concourse.bass_utils import run_bass_kernel_spmd

    nc = _get_nc()
    return run_bass_kernel_spmd(nc, in_maps, list(range(NCORES)), trace=trace, **kwargs)


def _numpy_fallback(X, Z, Wk, Wq, Wv, invr0, ptr):
    """Reference-exact fallback for ptr layouts other than 128 x 64."""
    X = np.asarray(X, dtype=np.float32)
    Z = np.asarray(Z, dtype=np.float32)
    n = X.shape[0]
    K = X @ Wk.T
    Q = X @ Wq.T
    V = X @ Wv.T
    seg = np.searchsorted(np.asarray(ptr)[1:], np.arange(n), side="right")
    out = np.zeros((n, Wk.shape[0]), dtype=np.float32)
    inv = float(np.asarray(invr0).reshape(-1)[0])
    hs = Wk.shape[0] ** -0.5
    for s in np.unique(seg):
        idx = np.nonzero(seg == s)[0]
        q, k, v, z = Q[idx], K[idx], V[idx], Z[idx]
        wei = (q @ k.T) * hs
        wei = wei - wei.max(axis=-1, keepdims=True)
        wei = np.exp(wei)
        wei /= wei.sum(axis=-1, keepdims=True)
        d2 = np.maximum(
            (z * z).sum(-1)[:, None] + (z * z).sum(-1)[None, :] - 2.0 * (z @ z.T), 0.0
        )
        dist = np.sqrt(np.where(d2 > 0, d2, 1.0)) * (d2 > 0)
        wei = wei * np.exp(-inv * dist)
        out[idx] = wei @ v
    return out


def kernel(X, Z, Wk, Wq, Wv, invr0, ptr):
    ptr = np.asarray(ptr)
    if not (
        X.shape == (N, E)
        and Wk.shape == (H, E)
        and ptr.shape == (NSEG + 1,)
        and np.array_equal(ptr, np.arange(NSEG + 1, dtype=ptr.dtype) * SEG)
    ):
        return _numpy_fallback(X, Z, Wk, Wq, Wv, invr0, ptr)

    in_maps = _prepare_in_maps(X, Z, Wk, Wq, Wv, invr0)
    res = _run(in_maps, trace=False)
    out = np.empty((N, H), dtype=np.float32)
    for d in range(NCORES):
        out[d * RPC : (d + 1) * RPC] = res.results[d]["y"]
    return out



# revision 4
# speedup vs baseline: 1.0885x; 1.0885x over previous
"""Trainium2 Bass kernel for nn_DistHead (block-diagonal molecule attention), v2.

out = softmax_blockdiag(Q K^T / sqrt(H)) * exp(-invr0 * cdist(Z, Z)) @ V
with Q/K/V = X @ W{q,k,v}^T, block-diagonal over 128 molecules of 64 atoms.

Sharding: 16 whole molecules (1024 rows) per core across 8 cores --
perfectly parallel, zero cross-core communication.

Design vs the previous version:
- Distance^2 matmul in fp16 with hi/lo-split coordinates (K=16) instead of
  double-pumped fp32; the block-diagonal mask is folded into this matmul as
  +-C rows so off-block v jumps by ~625 -> exp(-sqrt(v)) == 0 exactly in
  fp16.  An epsilon row keeps v > 0 so no max(v,0) pass is needed.
- Scores computed transposed (lhsT = K^T, rhs = Q^T): exp(s^T) is already in
  PV orientation -> no PE transposes, no identity, no psum->sbuf weight
  copies.  Softmax row sums come from a tiny N=2 matmul against block-ones
  columns.
- 3 input DMA triggers on 3 different engines (sync / scalar / gpsimd);
  ACT table order = Sqrt set once then Exp set once.
- fp16 output, upcast to fp32 on host.
"""

import sys

if "/opt/trn_rl_repo" not in sys.path:
    sys.path.insert(0, "/opt/trn_rl_repo")

import numpy as np

N, E, H = 8192, 256, 64          # atoms, embedding, head size
NSEG, SEG = 128, 64              # molecules, atoms per molecule
NCORES = 8
RPC = N // NCORES                # rows per core (1024 = 16 molecules)
NT = RPC // 128                  # 128-row tiles per core (2 molecules each)
EC = E // 128                    # embedding chunks of 128
KD = 16                          # contraction rows of the distance matmul

MASK_C = np.float16(17.68)       # mask row magnitude: off-block v += 2*C^2 ~ 625
EPS_A = np.float16(0.002)        # eps row: v += 4e-6 keeps sqrt input positive

_cache = {}


def _build_nc():
    import concourse.bacc as bacc
    import concourse.tile as tile
    from concourse import mybir

    f32 = mybir.dt.float32
    f16 = mybir.dt.float16
    AF = mybir.ActivationFunctionType

    nc = bacc.Bacc(None, target_bir_lowering=False, debug=False)

    # zz: [16, 2048] fp16, cols 0:1024 = A-side rows, 1024:2048 = B-side rows.
    zz_d = nc.dram_tensor("zz", [KD, 2 * RPC], f16, kind="ExternalInput")
    # wc: packed consts [128, 386] fp16:
    #   cols 0:256    = Wq^T*scale | Wk^T  per 128-chunk c: [128c..128c+64) q, [..+128) k
    #   cols 256:384  = Wv^T per chunk c at [256+64c, 256+64c+64)
    #   cols 384:386  = mask2 (block-ones columns for row sums)
    wc_d = nc.dram_tensor("wc", [128, 386], f16, kind="ExternalInput")
    xt_d = nc.dram_tensor("xt", [128, EC, RPC], f16, kind="ExternalInput")
    y_d = nc.dram_tensor("y", [RPC, H], f16, kind="ExternalOutput")

    with tile.TileContext(nc) as tc:
        with (
            tc.tile_pool(name="consts", bufs=1) as consts,
            tc.tile_pool(name="sb", bufs=1) as sb,
            tc.tile_pool(name="wide", bufs=1) as wide,
            tc.tile_pool(name="psbig", bufs=1, space="PSUM") as psbig,
            tc.tile_pool(name="psqk", bufs=2, space="PSUM") as psqk,
            tc.tile_pool(name="psv", bufs=1, space="PSUM") as psv,
            tc.tile_pool(name="pso", bufs=1, space="PSUM") as pso,
        ):
            # ---- input DMAs: one per engine ring, all fire immediately ----
            zz = consts.tile([KD, 2 * RPC], f16, tag="zz")
            nc.scalar.dma_start(out=zz, in_=zz_d[:, :])
            wcs = consts.tile([128, 386], f16, tag="wc")
            nc.gpsimd.dma_start(out=wcs, in_=wc_d[:, :])
            xt = consts.tile([128, EC, RPC], f16, tag="xt")
            nc.sync.dma_start(out=xt, in_=xt_d[:, :, :])

            # ---- distance pipeline (high priority: feeds the ACT chain) ----
            d_ps = psbig.tile([128, NT, 128], f32, tag="big")
            u = wide.tile([128, NT, 128], f32, tag="u")
            g = wide.tile([128, NT, 128], f16, tag="g")
            with tc.high_priority():
                for t in range(NT):
                    rt = slice(t * 128, (t + 1) * 128)
                    rtb = slice(RPC + t * 128, RPC + (t + 1) * 128)
                    nc.tensor.matmul(
                        d_ps[:, t, :], lhsT=zz[:, rt], rhs=zz[:, rtb],
                        start=True, stop=True,
                    )
                # v > 0 by construction (eps row) -> sqrt directly from psum.
                nc.scalar.activation(out=u, in_=d_ps, func=AF.Sqrt)
                nc.scalar.activation(out=g, in_=u, func=AF.Exp, scale=-1.0)

            # ---- Q/K projections -> K^T/Q^T in sbuf fp16 ----
            # ksb[h, j] = K[j, h], qsb[h, i] = scale*Q[i, h]
            ksb = sb.tile([H, RPC], f16, tag="ksb")
            qsb = sb.tile([H, RPC], f16, tag="qsb")
            for h in range(RPC // 512):
                cs = slice(h * 512, (h + 1) * 512)
                for iw, dst in ((0, qsb), (1, ksb)):
                    p = psqk.tile([H, 512], f32, tag="qk")
                    for c in range(EC):
                        nc.tensor.matmul(
                            p,
                            lhsT=wcs[:, 128 * c + 64 * iw : 128 * c + 64 * iw + 64],
                            rhs=xt[:, c, cs],
                            start=(c == 0), stop=(c == EC - 1),
                        )
                    nc.vector.tensor_copy(out=dst[:, cs], in_=p)

            # ---- V projection: v_sb[j, t, h] = V[128t+j, h] ----
            v_ps = psv.tile([128, NT, H], f32, tag="v")
            for t in range(NT):
                rt = slice(t * 128, (t + 1) * 128)
                for c in range(EC):
                    nc.tensor.matmul(
                        v_ps[:, t, :],
                        lhsT=xt[:, c, rt],
                        rhs=wcs[:, 256 + 64 * c : 256 + 64 * c + 64],
                        start=(c == 0), stop=(c == EC - 1),
                    )
            v_sb = sb.tile([128, NT, H], f16, tag="v_sb")
            nc.vector.tensor_copy(out=v_sb, in_=v_ps)

            # ---- scores^T: st[j, i] = k_j . q_i (already scaled) ----
            st_ps = psbig.tile([128, NT, 128], f32, tag="big")
            for t in range(NT):
                rt = slice(t * 128, (t + 1) * 128)
                nc.tensor.matmul(
                    st_ps[:, t, :], lhsT=ksb[:, rt], rhs=qsb[:, rt],
                    start=True, stop=True,
                )

            # ---- per-half: exp, decay-mask multiply, row sums, PV, scale ----
            HF = NT // 2
            et = wide.tile([128, NT, 128], f16, tag="et")
            weit = wide.tile([128, NT, 128], f16, tag="weit")
            rs_ps = pso.tile([128, NT, 2], f32, tag="rs")
            o_ps = pso.tile([128, NT, H], f32, tag="o")
            rinv = sb.tile([128, NT], f32, tag="rinv")
            o_sb = sb.tile([128, NT, H], f16, tag="o_sb")
            mask2 = wcs[:, 384:386]
            y_r = y_d.rearrange("(t p) h -> p t h", p=128)
            for hh in range(2):
                hs = slice(hh * HF, (hh + 1) * HF)
                nc.scalar.activation(out=et[:, hs, :], in_=st_ps[:, hs, :], func=AF.Exp)
                for t in range(hh * HF, (hh + 1) * HF):
                    nc.vector.tensor_mul(
                        out=weit[:, t, :], in0=et[:, t, :], in1=g[:, t, :]
                    )
                    nc.tensor.matmul(
                        rs_ps[:, t, :], lhsT=et[:, t, :], rhs=mask2,
                        start=True, stop=True,
                    )
                    nc.tensor.matmul(
                        o_ps[:, t, :], lhsT=weit[:, t, :], rhs=v_sb[:, t, :],
                        start=True, stop=True,
                    )
                # rows 0:64 of each tile sum block A (col 0), rows 64:128 block B.
                nc.vector.reciprocal(
                    out=rinv[0:64, hs], in_=rs_ps[0:64, hs, 0]
                )
                nc.vector.reciprocal(
                    out=rinv[64:128, hs], in_=rs_ps[64:128, hs, 1]
                )
                for t in range(hh * HF, (hh + 1) * HF):
                    nc.vector.tensor_scalar_mul(
                        out=o_sb[:, t, :], in0=o_ps[:, t, :],
                        scalar1=rinv[:, t : t + 1],
                    )
                eng = nc.sync if hh == 0 else nc.scalar
                eng.dma_start(out=y_r[:, hs, :], in_=o_sb[:, hs, :])

    nc.compile()
    return nc


def _get_nc():
    if "nc" not in _cache:
        _cache["nc"] = _build_nc()
    return _cache["nc"]


def _prepare_in_maps(X, Z, Wk, Wq, Wv, invr0):
    f16 = np.float16
    X = np.ascontiguousarray(X, dtype=np.float32)
    Z = np.ascontiguousarray(Z, dtype=np.float32)
    # [128, EC, N] fp16: partition p, chunk c -> X^T row c*128+p.
    xt_full = np.ascontiguousarray(
        X.T.reshape(EC, 128, N).transpose(1, 0, 2).astype(f16)
    )

    # invr0 folded into the coordinates: v = (invr0*dist)^2 (+mask/eps rows),
    # so the decay is exp(-1.0 * sqrt(v)).
    inv = np.float32(np.asarray(invr0).reshape(-1)[0])
    zs = (Z * inv).astype(np.float32)                     # [N, 3]
    z2s = np.sum(zs * zs, axis=-1)                        # [N]
    zh = zs.astype(f16)
    zl = (zs - zh.astype(np.float32)).astype(f16)
    z2h = z2s.astype(f16)
    z2l = (z2s - z2h.astype(np.float32)).astype(f16)
    ones = np.ones(N, dtype=f16)
    sig = np.where((np.arange(N) % 128) < SEG, 1.0, -1.0).astype(f16)

    # Mask rows FIRST: the +-C^2 pair cancels exactly at the head of the
    # sequential psum accumulation, keeping on-block noise at fp32 level.
    za = np.empty((KD, N), dtype=f16)
    zb = np.empty((KD, N), dtype=f16)
    za[0], zb[0] = MASK_C * ones, MASK_C * ones
    za[1], zb[1] = MASK_C * sig, -MASK_C * sig
    za[2], zb[2] = z2h, ones
    za[3], zb[3] = z2l, ones
    za[4], zb[4] = ones, z2h
    za[5], zb[5] = ones, z2l
    for d in range(3):
        za[6 + d], zb[6 + d] = -2.0 * zh[:, d], zh[:, d]
        za[9 + d], zb[9 + d] = -2.0 * zl[:, d], zh[:, d]
        za[12 + d], zb[12 + d] = -2.0 * zh[:, d], zl[:, d]
    za[15], zb[15] = EPS_A * ones, EPS_A * ones

    scale = np.float32(H) ** np.float32(-0.5)
    # wc: [128, 386] fp16 packed consts.
    wc = np.zeros((128, 386), dtype=f16)
    wqT = (Wq.T * scale).astype(np.float32).reshape(EC, 128, H)
    wkT = Wk.T.astype(np.float32).reshape(EC, 128, H)
    wvT = Wv.T.astype(np.float32).reshape(EC, 128, H)
    for c in range(EC):
        wc[:, 128 * c : 128 * c + 64] = wqT[c].astype(f16)
        wc[:, 128 * c + 64 : 128 * c + 128] = wkT[c].astype(f16)
        wc[:, 256 + 64 * c : 256 + 64 * c + 64] = wvT[c].astype(f16)
    wc[:, 384] = (np.arange(128) < 64).astype(f16)
    wc[:, 385] = (np.arange(128) >= 64).astype(f16)

    in_maps = []
    for d in range(NCORES):
        s, e = d * RPC, (d + 1) * RPC
        zz = np.concatenate([za[:, s:e], zb[:, s:e]], axis=1)
        in_maps.append(
            {
                "xt": np.ascontiguousarray(xt_full[:, :, s:e]),
                "zz": np.ascontiguousarray(zz),
                "wc": wc,
            }
        )
    return in_maps


def _run(in_maps, trace=False, **kwargs):
    from concourse.bass_utils import run_bass_kernel_spmd

    nc = _get_nc()
    return run_bass_kernel_spmd(nc, in_maps, list(range(NCORES)), trace=trace, **kwargs)


def _numpy_fallback(X, Z, Wk, Wq, Wv, invr0, ptr):
    """Reference-exact fallback for ptr layouts other than 128 x 64."""
    X = np.asarray(X, dtype=np.float32)
    Z = np.asarray(Z, dtype=np.float32)
    n = X.shape[0]
    K = X @ Wk.T
    Q = X @ Wq.T
    V = X @ Wv.T
    seg = np.searchsorted(np.asarray(ptr)[1:], np.arange(n), side="right")
    out = np.zeros((n, Wk.shape[0]), dtype=np.float32)
    inv = float(np.asarray(invr0).reshape(-1)[0])
    hs = Wk.shape[0] ** -0.5
    for s in np.unique(seg):
        idx = np.nonzero(seg == s)[0]
        q, k, v, z = Q[idx], K[idx], V[idx], Z[idx]
        wei = (q @ k.T) * hs
        wei = wei - wei.max(axis=-1, keepdims=True)
        wei = np.exp(wei)
        wei /= wei.sum(axis=-1, keepdims=True)
        d2 = np.maximum(
            (z * z).sum(-1)[:, None] + (z * z).sum(-1)[None, :] - 2.0 * (z @ z.T), 0.0
        )
        dist = np.sqrt(np.where(d2 > 0, d2, 1.0)) * (d2 > 0)
        wei = wei * np.exp(-inv * dist)
        out[idx] = wei @ v
    return out


def kernel(X, Z, Wk, Wq, Wv, invr0, ptr):
    ptr = np.asarray(ptr)
    if not (
        X.shape == (N, E)
        and Wk.shape == (H, E)
        and ptr.shape == (NSEG + 1,)
        and np.array_equal(ptr, np.arange(NSEG + 1, dtype=ptr.dtype) * SEG)
    ):
        return _numpy_fallback(X, Z, Wk, Wq, Wv, invr0, ptr)

    in_maps = _prepare_in_maps(X, Z, Wk, Wq, Wv, invr0)
    res = _run(in_maps, trace=False)
    out = np.empty((N, H), dtype=np.float32)
    for d in range(NCORES):
        out[d * RPC : (d + 1) * RPC] = res.results[d]["y"].astype(np.float32)
    return out


# revision 18
# speedup vs baseline: 1.1363x; 1.0439x over previous
"""Trainium2 Bass kernel for nn_DistHead (block-diagonal molecule attention), v5.

out = softmax_blockdiag(Q K^T / sqrt(H)) * exp(-invr0 * cdist(Z, Z)) @ V
with Q/K/V = X @ W{q,k,v}^T, block-diagonal over 128 molecules of 64 atoms.

Sharding: 16 whole molecules (1024 rows) per core across 8 cores --
perfectly parallel, zero cross-core communication.

Key structure:
- Block-diagonal mask folded into the score matmul as two augmented
  contraction rows (+-25 sigma / -625), so exp underflows off-block scores
  to exactly 0 in fp16.
- Scores computed transposed (lhsT = K^T, rhs = Q^T): exp(s^T) is already
  in PV orientation -> no PE transposes.  Softmax row sums come from an
  N=1 matmul against a ones column into the same psum tile as PV output.
- Distance^2 computed only for the on-block 64x64 molecule blocks, via a
  K=14 fp16 matmul with hi/lo-split coordinates; mol-A/mol-B use column
  groups (0,0)/(0,64), tile pairs (t, t+4) use row groups 0/32 with
  per-pair psum banks.  An epsilon row keeps v > 0.
- sqrt via exp(0.5*ln(v)): ln and exp live in one ACT table set
  (natural_log_exp_and_others), so the whole ACT chain needs one table
  load instead of three.
- PE warm-up matmuls run during the DMA wait so HAM unthrottles to
  2.4 GHz before the real matmuls arrive.
- X^T split into two column halves on the two HWDGE rings; distance and
  weight operands ride the SWDGE ring.  fp16 output, upcast on host.
"""

import sys

if "/opt/trn_rl_repo" not in sys.path:
    sys.path.insert(0, "/opt/trn_rl_repo")

import numpy as np

N, E, H = 8192, 256, 64          # atoms, embedding, head size
NSEG, SEG = 128, 64              # molecules, atoms per molecule
NCORES = 8
RPC = N // NCORES                # rows per core (1024 = 16 molecules)
NT = RPC // 128                  # 128-row tiles per core (2 molecules each)
HF = NT // 2
EC = E // 128                    # embedding chunks of 128
KD = 14                          # contraction rows of the distance matmul

AUG_S = np.float16(25.0)         # score mask rows: +-25 sigma, -625 bias
EPS_A = np.float16(0.002)        # eps row: v += 4e-6 keeps ln input positive
NWARM = 6                        # PE warm-up matmuls

_cache = {}


def _build_nc():
    import concourse.bacc as bacc
    import concourse.tile as tile
    from concourse import mybir

    f32 = mybir.dt.float32
    f16 = mybir.dt.float16
    AF = mybir.ActivationFunctionType

    nc = bacc.Bacc(None, target_bir_lowering=False, debug=False)

    # zz: [128, HF, 256] fp16.  Partitions 32g..32g+14 hold the distance rows
    # of tile pair (p, p+4); free: pair p, then [zaA|zaB|zbA|zbB] 64 each.
    zz_d = nc.dram_tensor("zz", [128, HF, 256], f16, kind="ExternalInput")
    # wc: packed consts [128, 386] fp16:
    #   cols 0:256   = Wq^T*scale | Wk^T per 128-chunk c
    #   cols 256:384 = Wv^T per chunk c
    #   col 384      = ones (row-sum matmul rhs), col 385 unused
    wc_d = nc.dram_tensor("wc", [128, 386], f16, kind="ExternalInput")
    # score-mask augmentation rows: [ones, 25*sig] for Q^T, [-625, 25*sig] for K^T
    aug_d = nc.dram_tensor("aug", [4, RPC], f16, kind="ExternalInput")
    # X^T fp16 split into two 512-col halves (one per HWDGE ring).
    xa_d = nc.dram_tensor("xa", [128, EC, 512], f16, kind="ExternalInput")
    xb_d = nc.dram_tensor("xb", [128, EC, 512], f16, kind="ExternalInput")
    y_d = nc.dram_tensor("y", [RPC, H], f16, kind="ExternalOutput")

    with tile.TileContext(nc) as tc:
        with (
            tc.tile_pool(name="consts", bufs=1) as consts,
            tc.tile_pool(name="sb", bufs=1) as sb,
            tc.tile_pool(name="wide", bufs=1) as wide,
            tc.tile_pool(name="psbig", bufs=1, space="PSUM") as psbig,
            tc.tile_pool(name="psst", bufs=1, space="PSUM") as psst,
            tc.tile_pool(name="psqk", bufs=2, space="PSUM") as psqk,
            tc.tile_pool(name="pso", bufs=1, space="PSUM") as pso,
        ):
            # ---- input DMAs: gpsimd(SWDGE) gets zz, wc, aug; sync/scalar
            # (the two HWDGE rings) get the two X^T halves ----
            ksb = sb.tile([H + 2, RPC], f16, tag="ksb")
            qsb = sb.tile([H + 2, RPC], f16, tag="qsb")
            zz = consts.tile([128, HF, 256], f16, tag="zz")
            nc.gpsimd.dma_start(out=zz, in_=zz_d[:, :, :])
            wcs = consts.tile([128, 386], f16, tag="wc")
            nc.gpsimd.dma_start(out=wcs, in_=wc_d[:, :])
            nc.gpsimd.dma_start(out=qsb[H : H + 2, :], in_=aug_d[0:2, :])
            nc.gpsimd.dma_start(out=ksb[H : H + 2, :], in_=aug_d[2:4, :])
            xa = consts.tile([128, EC, 512], f16, tag="xa")
            nc.sync.dma_start(out=xa, in_=xa_d[:, :, :])
            xb = consts.tile([128, EC, 512], f16, tag="xb")
            nc.scalar.dma_start(out=xb, in_=xb_d[:, :, :])
            xh = (xa, xb)

            # ---- PE warm-up: dummy matmuls over a memset scratch keep the
            # PE busy through the DMA wait so HAM unthrottles to 2.4 GHz. ----
            scratch = sb.tile([128, 512], f16, tag="scratch")
            nc.vector.memset(scratch, 0.0)
            warm_ps = psbig.tile([128, NT, 128], f32, tag="big")
            for i in range(NWARM):
                nc.tensor.matmul(
                    warm_ps[:, 4 * (i % 2) : 4 * (i % 2) + 4, :],
                    lhsT=scratch[:, 0:128], rhs=scratch,
                    start=True, stop=True,
                )

            # ---- distance pipeline (high priority: feeds the ACT chain) ----
            # d halves live in the score psum tiles (cols 0:64), version 1.
            d_ps = [
                psst.tile([128, HF, 128], f32, tag=f"st{i}", name=f"d{i}")
                for i in range(2)
            ]
            u = wide.tile([128, NT, H], f32, tag="u")
            g = wide.tile([128, NT, H], f16, tag="g")
            with tc.high_priority():
                for p in range(HF):
                    for mol, co in ((0, 0), (1, 64)):
                        for gi in range(2):  # row groups 0/32 = tiles p, p+4
                            nc.tensor.matmul(
                                d_ps[gi][64 * mol : 64 * mol + 64, p, 0:64],
                                lhsT=zz[32 * gi : 32 * gi + KD, p, co : co + 64],
                                rhs=zz[32 * gi : 32 * gi + KD, p, 128 + co : 192 + co],
                                start=True, stop=True,
                                tile_position=(32 * gi, co),
                            )
                # v > 0 by construction (eps row); sqrt(v) = exp(0.5*ln(v))
                # keeps every ACT op inside one table set (ln+exp).
                for i in range(2):
                    hs = slice(i * HF, (i + 1) * HF)
                    nc.scalar.activation(
                        out=u[:, hs, :], in_=d_ps[i][:, :, 0:64], func=AF.Ln
                    )
                    nc.scalar.activation(
                        out=u[:, hs, :], in_=u[:, hs, :], func=AF.Exp, scale=0.5
                    )
                    nc.scalar.activation(
                        out=g[:, hs, :], in_=u[:, hs, :], func=AF.Exp, scale=-1.0
                    )

            # ---- Q/K projections -> K^T/Q^T in sbuf fp16 ----
            for h in range(2):
                cs = slice(h * 512, (h + 1) * 512)
                for iw, dst in ((0, qsb), (1, ksb)):
                    p = psqk.tile([H, 512], f32, tag="qk")
                    for c in range(EC):
                        nc.tensor.matmul(
                            p,
                            lhsT=wcs[:, 128 * c + 64 * iw : 128 * c + 64 * iw + 64],
                            rhs=xh[h][:, c, :],
                            start=(c == 0), stop=(c == EC - 1),
                        )
                    nc.vector.tensor_copy(out=dst[0:H, cs], in_=p)

            # ---- V projection into the (freed) warm-up psum banks ----
            v_ps = psbig.tile([128, NT, 128], f32, tag="big")
            for t in range(NT):
                rt = slice((t % 4) * 128, (t % 4) * 128 + 128)
                for c in range(EC):
                    nc.tensor.matmul(
                        v_ps[:, t, 0:H],
                        lhsT=xh[t // 4][:, c, rt],
                        rhs=wcs[:, 256 + 64 * c : 256 + 64 * c + 64],
                        start=(c == 0), stop=(c == EC - 1),
                    )
            v_sb = sb.tile([128, NT, H], f16, tag="v_sb")
            nc.vector.tensor_copy(out=v_sb, in_=v_ps[:, :, 0:H])

            # ---- scores^T with mask rows: st[j, i] = k_j.q_i - 625*offblk ----
            st_ps = [
                psst.tile([128, HF, 128], f32, tag=f"st{i}", name=f"st{i}")
                for i in range(2)
            ]
            for t in range(NT):
                rt = slice(t * 128, (t + 1) * 128)
                nc.tensor.matmul(
                    st_ps[t // HF][:, t % HF, :], lhsT=ksb[:, rt], rhs=qsb[:, rt],
                    start=True, stop=True,
                )

            # ---- per-half: exp, on-block decay multiply, row sums, PV ----
            et = wide.tile([128, NT, 128], f16, tag="et")
            weit = wide.tile([128, NT, 128], f16, tag="weit")
            nc.vector.memset(weit, 0.0)
            oc_ps = [
                pso.tile([128, HF, 66], f32, tag=f"oc{i}", name=f"oc{i}")
                for i in range(2)
            ]
            rinv = sb.tile([128, NT], f32, tag="rinv")
            o_sb = sb.tile([128, NT, H], f16, tag="o_sb")
            ones_col = wcs[:, 384:385]
            y_r = y_d.rearrange("(t p) h -> p t h", p=128)

            for hh in range(2):
                nc.scalar.activation(
                    out=et[:, hh * HF : (hh + 1) * HF, :], in_=st_ps[hh],
                    func=AF.Exp,
                )
            for hh in range(2):
                hs = slice(hh * HF, (hh + 1) * HF)
                oc = oc_ps[hh]
                # et is exactly 0 off-block, so the decay multiply only needs
                # the two on-block quadrants; weit stays 0 elsewhere.
                nc.vector.tensor_mul(
                    out=weit[0:64, hs, 0:64], in0=et[0:64, hs, 0:64],
                    in1=g[0:64, hs, :],
                )
                nc.vector.tensor_mul(
                    out=weit[64:128, hs, 64:128], in0=et[64:128, hs, 64:128],
                    in1=g[64:128, hs, :],
                )
                for t in range(hh * HF, (hh + 1) * HF):
                    i = t % HF
                    nc.tensor.matmul(
                        oc[:, i, 64:65], lhsT=et[:, t, :], rhs=ones_col,
                        start=True, stop=True,
                    )
                    nc.tensor.matmul(
                        oc[:, i, 0:64], lhsT=weit[:, t, :], rhs=v_sb[:, t, :],
                        start=True, stop=True,
                    )
                nc.vector.reciprocal(out=rinv[:, hs], in_=oc[:, :, 64])
                for t in range(hh * HF, (hh + 1) * HF):
                    i = t % HF
                    if t % 4 >= 2:
                        nc.scalar.mul(
                            out=o_sb[:, t, :], in_=oc[:, i, 0:64],
                            mul=rinv[:, t : t + 1],
                        )
                    else:
                        nc.vector.tensor_scalar_mul(
                            out=o_sb[:, t, :], in0=oc[:, i, 0:64],
                            scalar1=rinv[:, t : t + 1],
                        )
                eng = nc.sync if hh == 0 else nc.scalar
                eng.dma_start(out=y_r[:, hs, :], in_=o_sb[:, hs, :])

    nc.compile()
    return nc


def _get_nc():
    if "nc" not in _cache:
        _cache["nc"] = _build_nc()
    return _cache["nc"]


def _prepare_in_maps(X, Z, Wk, Wq, Wv, invr0):
    f16 = np.float16
    X = np.ascontiguousarray(X, dtype=np.float32)
    Z = np.ascontiguousarray(Z, dtype=np.float32)
    # [128, EC, N] fp16: partition p, chunk c -> X^T row c*128+p.
    xt_full = np.ascontiguousarray(
        X.T.reshape(EC, 128, N).transpose(1, 0, 2).astype(f16)
    )

    # invr0 folded into the coordinates: v = (invr0*dist)^2 (+eps row),
    # so the decay is exp(-1.0 * sqrt(v)).
    inv = np.float32(np.asarray(invr0).reshape(-1)[0])
    zs = (Z * inv).astype(np.float32)                     # [N, 3]
    z2s = np.sum(zs * zs, axis=-1)                        # [N]
    zh = zs.astype(f16)
    zl = (zs - zh.astype(np.float32)).astype(f16)
    z2h = z2s.astype(f16)
    z2l = (z2s - z2h.astype(np.float32)).astype(f16)
    ones = np.ones(N, dtype=f16)

    za = np.empty((KD, N), dtype=f16)
    zb = np.empty((KD, N), dtype=f16)
    za[0], zb[0] = z2h, ones
    za[1], zb[1] = z2l, ones
    za[2], zb[2] = ones, z2h
    za[3], zb[3] = ones, z2l
    for d in range(3):
        za[4 + d], zb[4 + d] = -2.0 * zh[:, d], zh[:, d]
        za[7 + d], zb[7 + d] = -2.0 * zl[:, d], zh[:, d]
        za[10 + d], zb[10 + d] = -2.0 * zh[:, d], zl[:, d]
    za[13], zb[13] = EPS_A * ones, EPS_A * ones

    scale = np.float32(H) ** np.float32(-0.5)
    # wc: [128, 386] fp16 packed consts.
    wc = np.zeros((128, 386), dtype=f16)
    wqT = (Wq.T * scale).astype(np.float32).reshape(EC, 128, H)
    wkT = Wk.T.astype(np.float32).reshape(EC, 128, H)
    wvT = Wv.T.astype(np.float32).reshape(EC, 128, H)
    for c in range(EC):
        wc[:, 128 * c : 128 * c + 64] = wqT[c].astype(f16)
        wc[:, 128 * c + 64 : 128 * c + 128] = wkT[c].astype(f16)
        wc[:, 256 + 64 * c : 256 + 64 * c + 64] = wvT[c].astype(f16)
    wc[:, 384] = 1.0

    # score mask rows: on-block -625 + 625 = 0, off-block -1250 -> exp = 0.
    sig = np.where((np.arange(N) % 128) < SEG, 1.0, -1.0).astype(f16)
    aug_full = np.stack(
        [np.ones(N, f16), AUG_S * sig, np.full(N, -625.0, f16), AUG_S * sig]
    )

    in_maps = []
    for d in range(NCORES):
        s, e = d * RPC, (d + 1) * RPC
        # zz packed: row groups 0/32 <- tile pair (p, p+4); cols
        # [zaA | zaB | zbA | zbB] per 64-atom molecule block.
        zz = np.zeros((128, HF, 256), dtype=f16)
        for t in range(NT):
            gi, p = t // HF, t % HF
            for mol in range(2):
                ms = slice(s + t * 128 + 64 * mol, s + t * 128 + 64 * (mol + 1))
                zz[32 * gi : 32 * gi + KD, p, 64 * mol : 64 * mol + 64] = za[:, ms]
                zz[32 * gi : 32 * gi + KD, p, 128 + 64 * mol : 192 + 64 * mol] = zb[:, ms]
        in_maps.append(
            {
                "xa": np.ascontiguousarray(xt_full[:, :, s : s + 512]),
                "xb": np.ascontiguousarray(xt_full[:, :, s + 512 : e]),
                "zz": zz,
                "wc": wc,
                "aug": np.ascontiguousarray(aug_full[:, s:e]),
            }
        )
    return in_maps


def _run(in_maps, trace=False, **kwargs):
    from concourse.bass_utils import run_bass_kernel_spmd

    nc = _get_nc()
    return run_bass_kernel_spmd(nc, in_maps, list(range(NCORES)), trace=trace, **kwargs)


def _numpy_fallback(X, Z, Wk, Wq, Wv, invr0, ptr):
    """Reference-exact fallback for ptr layouts other than 128 x 64."""
    X = np.asarray(X, dtype=np.float32)
    Z = np.asarray(Z, dtype=np.float32)
    n = X.shape[0]
    K = X @ Wk.T
    Q = X @ Wq.T
    V = X @ Wv.T
    seg = np.searchsorted(np.asarray(ptr)[1:], np.arange(n), side="right")
    out = np.zeros((n, Wk.shape[0]), dtype=np.float32)
    inv = float(np.asarray(invr0).reshape(-1)[0])
    hs = Wk.shape[0] ** -0.5
    for s in np.unique(seg):
        idx = np.nonzero(seg == s)[0]
        q, k, v, z = Q[idx], K[idx], V[idx], Z[idx]
        wei = (q @ k.T) * hs
        wei = wei - wei.max(axis=-1, keepdims=True)
        wei = np.exp(wei)
        wei /= wei.sum(axis=-1, keepdims=True)
        d2 = np.maximum(
            (z * z).sum(-1)[:, None] + (z * z).sum(-1)[None, :] - 2.0 * (z @ z.T), 0.0
        )
        dist = np.sqrt(np.where(d2 > 0, d2, 1.0)) * (d2 > 0)
        wei = wei * np.exp(-inv * dist)
        out[idx] = wei @ v
    return out


def kernel(X, Z, Wk, Wq, Wv, invr0, ptr):
    ptr = np.asarray(ptr)
    if not (
        X.shape == (N, E)
        and Wk.shape == (H, E)
        and ptr.shape == (NSEG + 1,)
        and np.array_equal(ptr, np.arange(NSEG + 1, dtype=ptr.dtype) * SEG)
    ):
        return _numpy_fallback(X, Z, Wk, Wq, Wv, invr0, ptr)

    in_maps = _prepare_in_maps(X, Z, Wk, Wq, Wv, invr0)
    res = _run(in_maps, trace=False)
    out = np.empty((N, H), dtype=np.float32)
    for d in range(NCORES):
        out[d * RPC : (d + 1) * RPC] = res.results[d]["y"].astype(np.float32)
    return out
